# revision 1
# baseline (speedup 1.0000x reference)
"""Code2Vec kernel for 8 Trainium2 NeuronCores.

Strategy (data-parallel over batch):
  - Host folds the FC layer into the embedding tables:
      ctx @ fc_W.T = v1 @ (W1+W3).T + p @ W2.T
    so VA = value_table @ A + fc_b/2 and PB = path_table @ B + fc_b/2 are
    precomputed once on host; the device gathers rows of VA/PB and only needs
    an elementwise add (fused into the second gather DMA) + tanh.
  - Each core handles 512 batch rows (10240 tokens, 80 tiles of 128 tokens,
    token t = tile*128 + partition).
  - Attention pooling over R=20 is a PE matmul with 0/1 selection masks as the
    stationary operand; an extra "es" column yields the softmax denominators
    for free in the same accumulation.
  - Tag classification: v^T (PE-transposed) @ tag_table^T (host-transposed),
    softmax over 20000 via scalar-engine Exp with fused row-sum accumulation
    (no max subtraction needed: |logits| <= ~20, verified vs reference).
"""

import numpy as np

import concourse.bass as bass
import concourse.bacc as bacc
import concourse.mybir as mybir
import concourse.tile as tile
from concourse.bass_utils import run_bass_kernel_spmd

NCORES = 8
B = 4096
R = 20
E = 128
TV = 20000
VV = 150000
PV = 200000

BL = B // NCORES         # 512 batch rows per core
NTOK = BL * R            # 10240 tokens per core
NTILE = NTOK // 128      # 80 token tiles
GS = 2                   # token tiles per gather strip
NSTRIP = NTILE // GS     # 20 strips
NB = BL // 128           # 4 b-tiles per core
TPB = R                  # 20 token tiles per b-tile
EC = 2048                # output columns per E chunk
NEC = (TV + EC - 1) // EC  # 10 chunks (last = 1568)

F32 = mybir.dt.float32
F16 = mybir.dt.float16
I32 = mybir.dt.int32


def _body(nc, tc, aps):
    Alu = mybir.AluOpType
    Act = mybir.ActivationFunctionType
    va, pb, tagt, vidx, pidx, sel, attw, attb, ident, out = aps

    with (
        tc.tile_pool(name="const", bufs=1) as cpool,
        tc.tile_pool(name="gather", bufs=3) as gpool,
        tc.tile_pool(name="cwp", bufs=3) as cwpool,
        tc.tile_pool(name="small", bufs=2) as spool,
        tc.tile_pool(name="ebuf", bufs=10) as epool,
        tc.tile_pool(name="stg", bufs=2) as stgpool,
        tc.tile_pool(name="psv", bufs=2, space="PSUM") as psv,
        tc.tile_pool(name="psvt", bufs=2, space="PSUM") as psvt,
        tc.tile_pool(name="psc", bufs=2, space="PSUM") as psc,
    ):
        # ---- resident constants
        tag_sb = cpool.tile([128, TV], F32, tag="tag")
        nc.sync.dma_start(out=tag_sb[:], in_=tagt[:])
        sel_sb = cpool.tile([128, TPB * 128], F32, tag="sel")
        nc.sync.dma_start(out=sel_sb[:], in_=sel[:])
        attw_sb = cpool.tile([128, E], F32, tag="attw")
        nc.sync.dma_start(out=attw_sb[:], in_=attw[:])
        attb_sb = cpool.tile([128, 1], F32, tag="attb")
        nc.sync.dma_start(out=attb_sb[:], in_=attb[:])
        shift_sb = cpool.tile([128, 1], F32, tag="shift")
        nc.vector.memset(shift_sb[:], -10.0)
        ident_sb = cpool.tile([128, 128], F32, tag="ident")
        nc.sync.dma_start(out=ident_sb[:], in_=ident[:])
        vidx_sb = cpool.tile([128, NTILE], I32, tag="vidx")
        nc.sync.dma_start(out=vidx_sb[:], in_=vidx[:])
        pidx_sb = cpool.tile([128, NTILE], I32, tag="pidx")
        nc.sync.dma_start(out=pidx_sb[:], in_=pidx[:])

        strips = {}

        def make_strip(s):
            # gather VA rows; second gather adds PB rows in the DMA datapath.
            # NOTE: HW indirect DMA only honors one index per partition per
            # call ([128,1] offsets) — multi-index calls return garbage, so
            # emit one call per 128-token tile.
            g = gpool.tile([128, GS * 128], F32, tag="g")
            for j3 in range(GS):
                nc.gpsimd.indirect_dma_start(
                    out=g[:, j3 * 128 : (j3 + 1) * 128],
                    out_offset=None,
                    in_=va,
                    in_offset=bass.IndirectOffsetOnAxis(
                        ap=vidx_sb[:, s * GS + j3 : s * GS + j3 + 1], axis=0
                    ),
                )
            for j3 in range(GS):
                nc.gpsimd.indirect_dma_start(
                    out=g[:, j3 * 128 : (j3 + 1) * 128],
                    out_offset=None,
                    in_=pb,
                    in_offset=bass.IndirectOffsetOnAxis(
                        ap=pidx_sb[:, s * GS + j3 : s * GS + j3 + 1], axis=0
                    ),
                    compute_op=Alu.add,
                )
            # c = tanh(VA_g + PB_g) in place
            nc.scalar.activation(out=g[:], in_=g[:], func=Act.Tanh)
            # scores_j = sum_e c[:, j, e] * att_w[e]
            scr = spool.tile([128, GS], F32, tag="scr")
            ttrs = spool.tile([128, GS * 128], F32, tag="ttrs")
            nc.vector.tensor_tensor(
                out=ttrs[:].rearrange("p (g e) -> p g e", e=128),
                in0=g[:].rearrange("p (g e) -> p g e", e=128),
                in1=attw_sb[:].rearrange("p e -> p () e").to_broadcast(
                    [128, GS, 128]
                ),
                op=Alu.mult,
            )
            nc.vector.tensor_reduce(
                out=scr[:],
                in_=ttrs[:].rearrange("p (g e) -> p g e", e=128),
                axis=mybir.AxisListType.X,
                op=Alu.add,
            )
            es = spool.tile([128, GS], F32, tag="es")
            nc.scalar.activation(
                out=es[:], in_=scr[:], func=Act.Exp, bias=attb_sb[:, 0:1]
            )
            # cw[:, j, 0:128] = c * es ; cw[:, j, 128] = es
            cw = cwpool.tile([128, GS * 129], F32, tag="cw")
            cw3 = cw[:].rearrange("p (g x) -> p g x", x=129)
            c3 = g[:].rearrange("p (g x) -> p g x", x=128)
            es3 = es[:].unsqueeze(2)
            nc.vector.tensor_tensor(
                out=cw3[:, :, 0:128],
                in0=c3,
                in1=es3.to_broadcast([128, GS, 128]),
                op=Alu.mult,
            )
            nc.vector.tensor_copy(out=cw3[:, :, 128:129], in_=es3)
            return cw

        # ---- stage A/B: gather + tanh + attention pool -> v^T per b-tile
        vts = []
        for kb in range(NB):
            pv = psv.tile([128, 129], F32, tag="pv")
            for jl in range(TPB):
                j = kb * TPB + jl
                s, j2 = divmod(j, GS)
                if s not in strips:
                    strips[s] = make_strip(s)
                nc.tensor.matmul(
                    out=pv[:],
                    lhsT=sel_sb[:, jl * 128 : (jl + 1) * 128],
                    rhs=strips[s][:, j2 * 129 : (j2 + 1) * 129],
                    start=(jl == 0),
                    stop=(jl == TPB - 1),
                )
            rs = spool.tile([128, 1], F32, tag="rs")
            rsl = spool.tile([128, 1], F32, tag="rsl")
            nc.scalar.activation(out=rsl[:], in_=pv[:, 128:129], func=Act.Ln)
            nc.scalar.activation(out=rs[:], in_=rsl[:], func=Act.Exp, scale=-1.0)
            vsb = spool.tile([128, 128], F32, tag="vsb")
            nc.vector.tensor_scalar_mul(out=vsb[:], in0=pv[:, 0:128], scalar1=rs[:])
            pvt = psvt.tile([128, 128], F32, tag="pvt")
            nc.tensor.transpose(out=pvt[:], in_=vsb[:], identity=ident_sb[:])
            vt = cpool.tile([128, 128], F32, tag=f"vt{kb}")
            nc.scalar.copy(out=vt[:], in_=pvt[:])
            vts.append(vt)

        # ---- stage C: logits = v^T.T @ tagT, softmax over 20000
        for kb in range(NB):
            sums = spool.tile([128, 2 * NEC], F32, tag=f"sums{kb}")
            echunks = []
            for gi in range(NEC):
                off = gi * EC
                w = min(EC, TV - off)
                et = epool.tile([128, EC], F16, tag="E")
                for h in range(2):
                    hoff = h * 1024
                    if hoff >= w:
                        break
                    wh = min(1024, w - hoff)
                    pc = psc.tile([128, 1024], F32, tag="pc")
                    for q in range(0, wh, 512):
                        wq = min(512, wh - q)
                        nc.tensor.matmul(
                            out=pc[:, q : q + wq],
                            lhsT=vts[kb][:],
                            rhs=tag_sb[:, off + hoff + q : off + hoff + q + wq],
                            start=True,
                            stop=True,
                        )
                    # fp16 E with constant -10 logit shift (cancelled by the
                    # normalization): exp(l-10) <= e^7 fits fp16 range.
                    nc.scalar.activation(
                        out=et[:, hoff : hoff + wh],
                        in_=pc[:, 0:wh],
                        func=Act.Exp,
                        bias=shift_sb[:, 0:1],
                        accum_out=sums[:, 2 * gi + h : 2 * gi + h + 1],
                    )
                echunks.append((et, off, w))
            stot = spool.tile([128, 1], F32, tag=f"stot{kb}")
            nc.vector.reduce_sum(out=stot[:], in_=sums[:], axis=mybir.AxisListType.X)
            rstot = spool.tile([128, 1], F32, tag=f"rst{kb}")
            rstl = spool.tile([128, 1], F32, tag=f"rstl{kb}")
            nc.scalar.activation(out=rstl[:], in_=stot[:], func=Act.Ln)
            nc.scalar.activation(out=rstot[:], in_=rstl[:], func=Act.Exp, scale=-1.0)
            for et, off, w in echunks:
                stg = stgpool.tile([128, EC], F32, tag="stg")
                nc.vector.tensor_scalar_mul(
                    out=stg[:, 0:w], in0=et[:, 0:w], scalar1=rstot[:]
                )
                nc.sync.dma_start(
                    out=out[kb * 128 : (kb + 1) * 128, off : off + w],
                    in_=stg[:, 0:w],
                )


def _build_program():
    nc = bacc.Bacc(
        "TRN2",
        target_bir_lowering=False,
        debug=False,
        num_devices=NCORES,
        # default 16KB SWDGE descriptor ring overflows (HW corruption/crash)
        # with our pipelined 128-row indirect gathers; 64KB is probe-validated.
        dynamic_dma_scratch_size=65536,
    )
    aps = (
        nc.dram_tensor("va_table", [VV, E], F32, kind="ExternalInput").ap(),
        nc.dram_tensor("pb_table", [PV, E], F32, kind="ExternalInput").ap(),
        nc.dram_tensor("tag_t", [E, TV], F32, kind="ExternalInput").ap(),
        nc.dram_tensor("v_idx", [128, NTILE], I32, kind="ExternalInput").ap(),
        nc.dram_tensor("p_idx", [128, NTILE], I32, kind="ExternalInput").ap(),
        nc.dram_tensor("sel", [128, TPB * 128], F32, kind="ExternalInput").ap(),
        nc.dram_tensor("att_wb", [128, E], F32, kind="ExternalInput").ap(),
        nc.dram_tensor("att_bb", [128, 1], F32, kind="ExternalInput").ap(),
        nc.dram_tensor("ident", [128, 128], F32, kind="ExternalInput").ap(),
        nc.dram_tensor("out", [BL, TV], F32, kind="ExternalOutput").ap(),
    )
    with tile.TileContext(nc) as tc:
        _body(nc, tc, aps)
    nc.compile()
    return nc


_NC_CACHE = None


def _get_program():
    global _NC_CACHE
    if _NC_CACHE is None:
        _NC_CACHE = _build_program()
    return _NC_CACHE


def _install_neff_cache():
    """Cache compiled NEFFs by BIR hash — the stock bass_exec path recompiles
    (~6 min) on every fresh process even for an identical program."""
    import hashlib
    import os
    import shutil

    import concourse.bass2jax as b2j
    import concourse.bass_utils as bu

    if getattr(bu, "_c2v_neff_cache", False):
        return
    orig = bu.compile_bir_kernel

    def cached(bir_json, tmpdir, neff_name="file.neff"):
        h = hashlib.sha256(bir_json).hexdigest()[:24]
        cdir = os.path.expanduser("~/.c2v_neff_cache")
        os.makedirs(cdir, exist_ok=True)
        cpath = os.path.join(cdir, h + ".neff")
        dst = os.path.join(tmpdir, neff_name)
        if os.path.exists(cpath):
            shutil.copyfile(cpath, dst)
            return dst
        p = orig(bir_json, tmpdir, neff_name)
        try:
            shutil.copyfile(p, cpath)
        except OSError:
            pass
        return p

    bu.compile_bir_kernel = cached
    b2j.compile_bir_kernel = cached
    bu._c2v_neff_cache = True


_install_neff_cache()


def _ensure_ntff_hook():
    """The agent image's antenv lacks axon_hooks; recreate it via ctypes on
    the injected libaxon_pjrt.so so trace=True produces NTFF profiles."""
    import contextlib
    import ctypes
    import sys
    import types

    try:
        from antenv.axon_hooks import get_axon_ntff_profile_hook  # noqa: F401

        return
    except ImportError:
        pass

    so_path = "/opt/axon/libaxon_pjrt.so"
    lib = ctypes.CDLL(so_path)
    hook = None
    if hasattr(lib, "axon_start_nrt_profile"):
        lib.axon_start_nrt_profile.argtypes = [
            ctypes.POINTER(ctypes.c_int64),
            ctypes.c_size_t,
        ]
        lib.axon_start_nrt_profile.restype = ctypes.c_int64
        lib.axon_stop_nrt_profile.argtypes = [ctypes.c_char_p]
        lib.axon_stop_nrt_profile.restype = ctypes.c_int64

        @contextlib.contextmanager
        def _hook(output_dir, device_ids):
            import jax

            jax.devices()
            if device_ids:
                ids = (ctypes.c_int64 * len(device_ids))(*device_ids)
                rc = lib.axon_start_nrt_profile(ids, len(device_ids))
            else:
                rc = lib.axon_start_nrt_profile(None, 0)
            if rc != 0:
                raise RuntimeError(f"axon_start_nrt_profile rc={rc}")
            try:
                yield
            finally:
                n = lib.axon_stop_nrt_profile(str(output_dir).encode())
                print(f"ntff profile: {n} file(s) written to {output_dir}")

        hook = _hook

    mod = types.ModuleType("antenv.axon_hooks")
    mod._hook = hook
    mod.get_axon_ntff_profile_hook = lambda: mod._hook
    mod.set_axon_ntff_profile_hook = lambda h: setattr(mod, "_hook", h)
    sys.modules["antenv.axon_hooks"] = mod
    import antenv

    antenv.axon_hooks = mod


def prep_in_maps(inputs):
    """Host-side input prep: fold FC into tables, transpose tag table,
    build selection masks, relayout indices; shard batch across cores."""
    v1i = np.asarray(inputs["value1_idx"]).astype(np.int32)
    pti = np.asarray(inputs["path_idx"]).astype(np.int32)
    vt = np.asarray(inputs["value_table"], dtype=np.float32)
    pt = np.asarray(inputs["path_table"], dtype=np.float32)
    tt = np.asarray(inputs["tag_table"], dtype=np.float32)
    fw = np.asarray(inputs["fc_W"], dtype=np.float64)
    fb = np.asarray(inputs["fc_b"], dtype=np.float64)
    aw = np.asarray(inputs["att_w"], dtype=np.float32)
    ab = np.float32(np.asarray(inputs["att_b"]))

    A = (fw[:, :E] + fw[:, 2 * E : 3 * E]).T  # [e_in, e_out]
    Bm = fw[:, E : 2 * E].T
    va = (vt.astype(np.float64) @ A + 0.5 * fb).astype(np.float32)
    pbt = (pt.astype(np.float64) @ Bm + 0.5 * fb).astype(np.float32)

    selm = np.zeros((128, TPB * 128), np.float32)
    p = np.arange(128)
    for j in range(TPB):
        bloc = (128 * j + p) // R
        selm[p, j * 128 + bloc] = 1.0

    common = dict(
        va_table=np.ascontiguousarray(va),
        pb_table=np.ascontiguousarray(pbt),
        tag_t=np.ascontiguousarray(tt.T),
        sel=selm,
        att_wb=np.ascontiguousarray(np.tile(aw[None, :], (128, 1))),
        att_bb=np.full((128, 1), ab, np.float32),
        ident=np.eye(128, dtype=np.float32),
    )
    in_maps = []
    for k in range(NCORES):
        vflat = v1i[k * BL : (k + 1) * BL, :].reshape(-1)  # token t = b*R + r
        pflat = pti[k * BL : (k + 1) * BL, :].reshape(-1)
        in_maps.append(
            dict(
                common,
                v_idx=np.ascontiguousarray(vflat.reshape(NTILE, 128).T),
                p_idx=np.ascontiguousarray(pflat.reshape(NTILE, 128).T),
            )
        )
    return in_maps


def run(inputs, trace=False, tmpdir=None):
    if trace:
        _ensure_ntff_hook()
    in_maps = prep_in_maps(inputs)
    nc = _get_program()
    res = run_bass_kernel_spmd(
        nc,
        in_maps,
        core_ids=list(range(NCORES)),
        trace=trace,
        tmpdir=tmpdir,
    )
    out = np.concatenate([res.results[k]["out"] for k in range(NCORES)], axis=0)
    return out, res


def kernel(**inputs) -> np.ndarray:
    out, _ = run(inputs, trace=False)
    return out



# revision 6
# speedup vs baseline: 1.0645x; 1.0645x over previous
"""Code2Vec kernel for 8 Trainium2 NeuronCores.

Strategy (data-parallel over batch, fp16 data path):
  - Host folds the FC layer into the embedding tables:
      ctx @ fc_W.T = v1 @ (W1+W3).T + p @ W2.T
    so VA = value_table @ A + fc_b/2 and PB = path_table @ B + fc_b/2.
  - Host COMPACTS the tables per core: only the <=10240 rows a core's
    tokens reference are shipped (np.unique remap). The compacted row ids
    fit int16, which unlocks the ucode dma_gather instruction: one gpsimd
    instruction gathers 2560 rows (vs. 128 rows per indirect_dma_start),
    eliminating the ~1us/call SWDGE fixed cost that dominated the gather
    phase, and the 2.6MB footprint gives near-sequential HBM locality.
  - Everything flows in fp16 (tables, tanh, attention, tag matmul inputs,
    output store); PE matmuls run 4x faster than the fp32 2-pass path and
    the 41MB output write halves. PSUM accumulation stays fp32. Host
    upcasts the final output to fp32. Measured rel err ~6e-4 (gate 2e-2).
  - Attention pooling over R=20 is a PE matmul with 0/1 selection masks
    (compile-time pattern) as the stationary operand; an extra "es" column
    yields the softmax denominators in the same accumulation; 1/sum via
    DVE reciprocal so the scalar engine only ever needs the exp/tanh
    activation table set (no table reloads).
  - Tag classification: v^T (PE-transposed) @ tag_table^T (host-transposed)
    in fp16, softmax over 20000 via scalar-engine Exp (bias -10, fp16 out)
    with fused row-sum accumulation, DVE normalize, fp16 store.
"""

import numpy as np

import concourse.bass as bass
import concourse.bacc as bacc
import concourse.mybir as mybir
import concourse.tile as tile
from concourse.bass_utils import run_bass_kernel_spmd

NCORES = 8
B = 4096
R = 20
E = 128
TV = 20000
VV = 150000
PV = 200000

BL = B // NCORES         # 512 batch rows per core
NTOK = BL * R            # 10240 tokens per core
NTILE = NTOK // 128      # 80 token tiles
NB = BL // 128           # 4 b-tiles per core
TPB = R                  # 20 token tiles per b-tile
QTOK = NTOK // NB        # 2560 tokens per b-tile quarter
EC = 2048                # output columns per chunk
NEC = (TV + EC - 1) // EC  # 10 chunks (last = 1568)

F32 = mybir.dt.float32
F16 = mybir.dt.float16
I16 = mybir.dt.int16

IDXC = QTOK // 16        # 160 idx columns per gather call


def _body(nc, tc, aps):
    Alu = mybir.AluOpType
    Act = mybir.ActivationFunctionType
    va, pb, tagt, vidx, pidx, sel, attw, attb, ident, out = aps

    with (
        tc.tile_pool(name="const", bufs=1) as cpool,
    ):
        # ---- resident constants
        tag_sb = cpool.tile([128, TV], F16, tag="tag")
        nc.sync.dma_start(out=tag_sb[:], in_=tagt[:])
        sel_sb = cpool.tile([128, TPB * 128], F16, tag="sel")
        nc.sync.dma_start(out=sel_sb[:], in_=sel[:])
        attw_sb = cpool.tile([128, E], F16, tag="attw")
        nc.sync.dma_start(out=attw_sb[:], in_=attw[:])
        attb_sb = cpool.tile([128, 1], F32, tag="attb")
        nc.sync.dma_start(out=attb_sb[:], in_=attb[:])
        shift_sb = cpool.tile([128, 1], F32, tag="shift")
        nc.vector.memset(shift_sb[:], -10.0)
        ident_sb = cpool.tile([128, 128], F16, tag="ident")
        nc.sync.dma_start(out=ident_sb[:], in_=ident[:])
        vidx_sb = cpool.tile([128, NB * IDXC], I16, tag="vidx")
        nc.sync.dma_start(out=vidx_sb[:], in_=vidx[:])
        pidx_sb = cpool.tile([128, NB * IDXC], I16, tag="pidx")
        nc.sync.dma_start(out=pidx_sb[:], in_=pidx[:])

        vts = []

        # ---- stage A: gather + tanh + attention pool -> v^T per b-tile
        with (
            tc.tile_pool(name="gva", bufs=2) as gpool,
            tc.tile_pool(name="gpb", bufs=2) as hpool,
            tc.tile_pool(name="cbuf", bufs=2) as cbpool,
            tc.tile_pool(name="cwp", bufs=2) as cwpool,
            tc.tile_pool(name="small", bufs=2) as spool,
            tc.tile_pool(name="psv", bufs=2, space="PSUM") as psv,
            tc.tile_pool(name="psvt", bufs=2, space="PSUM") as psvt,
        ):
            for q in range(NB):
                g = gpool.tile([128, QTOK], F16, tag="g")
                h = hpool.tile([128, QTOK], F16, tag="h")
                nc.gpsimd.dma_gather(
                    g[:].rearrange("p (t e) -> p t e", e=128),
                    va,
                    vidx_sb[:, q * IDXC : (q + 1) * IDXC],
                    QTOK,
                    QTOK,
                    E,
                    single_packet=False,
                )
                nc.gpsimd.dma_gather(
                    h[:].rearrange("p (t e) -> p t e", e=128),
                    pb,
                    pidx_sb[:, q * IDXC : (q + 1) * IDXC],
                    QTOK,
                    QTOK,
                    E,
                    single_packet=False,
                )
                c = cbpool.tile([128, QTOK], F16, tag="c")
                nc.vector.tensor_tensor(out=c[:], in0=g[:], in1=h[:], op=Alu.add)
                # c = tanh(VA_g + PB_g) in place
                nc.scalar.activation(out=c[:], in_=c[:], func=Act.Tanh)
                # scores_j = sum_e c[:, j, e] * att_w[e]
                scr = spool.tile([128, TPB], F32, tag="scr")
                ttrs = cbpool.tile([128, QTOK], F16, tag="ttrs")
                c3 = c[:].rearrange("p (g e) -> p g e", e=128)
                nc.vector.tensor_tensor(
                    out=ttrs[:].rearrange("p (g e) -> p g e", e=128),
                    in0=c3,
                    in1=attw_sb[:].rearrange("p e -> p () e").to_broadcast(
                        [128, TPB, 128]
                    ),
                    op=Alu.mult,
                )
                nc.vector.tensor_reduce(
                    out=scr[:],
                    in_=ttrs[:].rearrange("p (g e) -> p g e", e=128),
                    axis=mybir.AxisListType.X,
                    op=Alu.add,
                )
                es = spool.tile([128, TPB], F16, tag="es")
                nc.scalar.activation(
                    out=es[:], in_=scr[:], func=Act.Exp, bias=attb_sb[:, 0:1]
                )
                # cw[:, j, 0:128] = c * es ; cw[:, j, 128] = es
                cw = cwpool.tile([128, TPB * 129], F16, tag="cw")
                cw3 = cw[:].rearrange("p (g x) -> p g x", x=129)
                es3 = es[:].unsqueeze(2)
                nc.vector.tensor_tensor(
                    out=cw3[:, :, 0:128],
                    in0=c3,
                    in1=es3.to_broadcast([128, TPB, 128]),
                    op=Alu.mult,
                )
                nc.vector.tensor_copy(out=cw3[:, :, 128:129], in_=es3)

                pv = psv.tile([128, 129], F32, tag="pv")
                for jl in range(TPB):
                    nc.tensor.matmul(
                        out=pv[:],
                        lhsT=sel_sb[:, jl * 128 : (jl + 1) * 128],
                        rhs=cw3[:, jl, :],
                        start=(jl == 0),
                        stop=(jl == TPB - 1),
                    )
                rs = spool.tile([128, 1], F32, tag="rs")
                nc.vector.reciprocal(out=rs[:], in_=pv[:, 128:129])
                vsb = spool.tile([128, 128], F16, tag="vsb")
                nc.vector.tensor_scalar_mul(
                    out=vsb[:], in0=pv[:, 0:128], scalar1=rs[:]
                )
                pvt = psvt.tile([128, 128], F16, tag="pvt")
                nc.tensor.transpose(out=pvt[:], in_=vsb[:], identity=ident_sb[:])
                vt = cpool.tile([128, 128], F16, tag=f"vt{q}")
                nc.scalar.copy(out=vt[:], in_=pvt[:])
                vts.append(vt)

        # ---- stage C: logits = v^T.T @ tagT, softmax over 20000, fp16 store
        with (
            tc.tile_pool(name="ebuf", bufs=2) as epool,
            tc.tile_pool(name="stg", bufs=3) as stgpool,
            tc.tile_pool(name="sums", bufs=2) as supool,
            tc.tile_pool(name="psc", bufs=2, space="PSUM") as psc,
        ):
            for kb in range(NB):
                sums = supool.tile([128, NEC], F32, tag="sums")
                et = epool.tile([128, NEC * EC], F16, tag="et")
                for gi in range(NEC):
                    off = gi * EC
                    w = min(EC, TV - off)
                    pc = psc.tile([128, EC], F32, tag="pc")
                    for q in range(0, w, 512):
                        wq = min(512, w - q)
                        nc.tensor.matmul(
                            out=pc[:, q : q + wq],
                            lhsT=vts[kb][:],
                            rhs=tag_sb[:, off + q : off + q + wq],
                            start=True,
                            stop=True,
                        )
                    # fp16 exp with constant -10 logit shift (cancelled by the
                    # normalization): exp(l-10) <= e^7 fits fp16 range.
                    nc.scalar.activation(
                        out=et[:, off : off + w],
                        in_=pc[:, 0:w],
                        func=Act.Exp,
                        bias=shift_sb[:, 0:1],
                        accum_out=sums[:, gi : gi + 1],
                    )
                stot = supool.tile([128, 1], F32, tag="stot")
                nc.vector.reduce_sum(
                    out=stot[:], in_=sums[:], axis=mybir.AxisListType.X
                )
                rstot = supool.tile([128, 1], F32, tag="rstot")
                nc.vector.reciprocal(out=rstot[:], in_=stot[:])
                for gi in range(NEC):
                    off = gi * EC
                    w = min(EC, TV - off)
                    stg = stgpool.tile([128, EC], F16, tag="stg")
                    nc.vector.tensor_scalar_mul(
                        out=stg[:, 0:w], in0=et[:, off : off + w], scalar1=rstot[:]
                    )
                    nc.sync.dma_start(
                        out=out[kb * 128 : (kb + 1) * 128, off : off + w],
                        in_=stg[:, 0:w],
                    )


def _build_program():
    nc = bacc.Bacc(
        "TRN2",
        target_bir_lowering=False,
        debug=False,
        num_devices=NCORES,
        # SWDGE descriptor ring (SBUF carveout): dma_gather pushes one desc
        # pair per row; 64KB holds a 2560-row call's descriptors and is
        # probe-validated vs the 16KB default's overflow corruption.
        dynamic_dma_scratch_size=65536,
    )
    aps = (
        nc.dram_tensor("va_used", [NTOK, E], F16, kind="ExternalInput").ap(),
        nc.dram_tensor("pb_used", [NTOK, E], F16, kind="ExternalInput").ap(),
        nc.dram_tensor("tag_t", [E, TV], F16, kind="ExternalInput").ap(),
        nc.dram_tensor("v_idx", [128, NB * IDXC], I16, kind="ExternalInput").ap(),
        nc.dram_tensor("p_idx", [128, NB * IDXC], I16, kind="ExternalInput").ap(),
        nc.dram_tensor("sel", [128, TPB * 128], F16, kind="ExternalInput").ap(),
        nc.dram_tensor("att_wb", [128, E], F16, kind="ExternalInput").ap(),
        nc.dram_tensor("att_bb", [128, 1], F32, kind="ExternalInput").ap(),
        nc.dram_tensor("ident", [128, 128], F16, kind="ExternalInput").ap(),
        nc.dram_tensor("out", [BL, TV], F16, kind="ExternalOutput").ap(),
    )
    with tile.TileContext(nc) as tc:
        _body(nc, tc, aps)
    nc.compile()
    return nc


_NC_CACHE = None


def _get_program():
    global _NC_CACHE
    if _NC_CACHE is None:
        _NC_CACHE = _build_program()
    return _NC_CACHE


def _install_neff_cache():
    """Cache compiled NEFFs by BIR hash — the stock bass_exec path recompiles
    (~6 min) on every fresh process even for an identical program."""
    import hashlib
    import os
    import shutil

    import concourse.bass2jax as b2j
    import concourse.bass_utils as bu

    if getattr(bu, "_c2v_neff_cache", False):
        return
    orig = bu.compile_bir_kernel

    def cached(bir_json, tmpdir, neff_name="file.neff"):
        h = hashlib.sha256(bir_json).hexdigest()[:24]
        cdir = os.path.expanduser("~/.c2v_neff_cache")
        os.makedirs(cdir, exist_ok=True)
        cpath = os.path.join(cdir, h + ".neff")
        dst = os.path.join(tmpdir, neff_name)
        if os.path.exists(cpath):
            shutil.copyfile(cpath, dst)
            return dst
        p = orig(bir_json, tmpdir, neff_name)
        try:
            shutil.copyfile(p, cpath)
        except OSError:
            pass
        return p

    bu.compile_bir_kernel = cached
    b2j.compile_bir_kernel = cached
    bu._c2v_neff_cache = True


_install_neff_cache()


def _ensure_ntff_hook():
    """The agent image's antenv lacks axon_hooks; recreate it via ctypes on
    the injected libaxon_pjrt.so so trace=True produces NTFF profiles."""
    import contextlib
    import ctypes
    import sys
    import types

    try:
        from antenv.axon_hooks import get_axon_ntff_profile_hook  # noqa: F401

        return
    except ImportError:
        pass

    so_path = "/opt/axon/libaxon_pjrt.so"
    lib = ctypes.CDLL(so_path)
    hook = None
    if hasattr(lib, "axon_start_nrt_profile"):
        lib.axon_start_nrt_profile.argtypes = [
            ctypes.POINTER(ctypes.c_int64),
            ctypes.c_size_t,
        ]
        lib.axon_start_nrt_profile.restype = ctypes.c_int64
        lib.axon_stop_nrt_profile.argtypes = [ctypes.c_char_p]
        lib.axon_stop_nrt_profile.restype = ctypes.c_int64

        @contextlib.contextmanager
        def _hook(output_dir, device_ids):
            import jax

            jax.devices()
            if device_ids:
                ids = (ctypes.c_int64 * len(device_ids))(*device_ids)
                rc = lib.axon_start_nrt_profile(ids, len(device_ids))
            else:
                rc = lib.axon_start_nrt_profile(None, 0)
            if rc != 0:
                raise RuntimeError(f"axon_start_nrt_profile rc={rc}")
            try:
                yield
            finally:
                n = lib.axon_stop_nrt_profile(str(output_dir).encode())
                print(f"ntff profile: {n} file(s) written to {output_dir}")

        hook = _hook

    mod = types.ModuleType("antenv.axon_hooks")
    mod._hook = hook
    mod.get_axon_ntff_profile_hook = lambda: mod._hook
    mod.set_axon_ntff_profile_hook = lambda h: setattr(mod, "_hook", h)
    sys.modules["antenv.axon_hooks"] = mod
    import antenv

    antenv.axon_hooks = mod


def _wrap_idx(idx):
    """[NTOK] int -> [128, NB*IDXC] int16 in dma_gather's wrapped layout:
    call q's token i lives at [i % 16, q*IDXC + i // 16], 16-row block
    replicated 8x down the partition dim."""
    blocks = []
    for q in range(NB):
        blk = idx[q * QTOK : (q + 1) * QTOK].reshape(IDXC, 16).T  # [16, IDXC]
        blocks.append(blk)
    one = np.concatenate(blocks, axis=1)  # [16, NB*IDXC]
    return np.ascontiguousarray(np.tile(one, (8, 1)).astype(np.int16))


def prep_in_maps(inputs):
    """Host-side input prep: fold FC into tables, compact tables to the rows
    each core actually uses (int16-indexable), transpose tag table, build
    selection masks; shard batch across cores."""
    v1i = np.asarray(inputs["value1_idx"]).astype(np.int64)
    pti = np.asarray(inputs["path_idx"]).astype(np.int64)
    vt = np.asarray(inputs["value_table"], dtype=np.float32)
    pt = np.asarray(inputs["path_table"], dtype=np.float32)
    tt = np.asarray(inputs["tag_table"], dtype=np.float32)
    fw = np.asarray(inputs["fc_W"], dtype=np.float64)
    fb = np.asarray(inputs["fc_b"], dtype=np.float64)
    aw = np.asarray(inputs["att_w"], dtype=np.float32)
    ab = np.float32(np.asarray(inputs["att_b"]))

    A = (fw[:, :E] + fw[:, 2 * E : 3 * E]).T  # [e_in, e_out]
    Bm = fw[:, E : 2 * E].T

    selm = np.zeros((128, TPB * 128), np.float16)
    p = np.arange(128)
    for j in range(TPB):
        bloc = (128 * j + p) // R
        selm[p, j * 128 + bloc] = 1.0

    common = dict(
        tag_t=np.ascontiguousarray(tt.T.astype(np.float16)),
        sel=selm,
        att_wb=np.ascontiguousarray(
            np.tile(aw[None, :].astype(np.float16), (128, 1))
        ),
        att_bb=np.full((128, 1), ab, np.float32),
        ident=np.eye(128, dtype=np.float16),
    )
    in_maps = []
    for k in range(NCORES):
        vtok = v1i[k * BL : (k + 1) * BL, :].reshape(-1)  # token t = b*R + r
        ptok = pti[k * BL : (k + 1) * BL, :].reshape(-1)
        vu, vinv = np.unique(vtok, return_inverse=True)
        pu, pinv = np.unique(ptok, return_inverse=True)
        va_used = np.zeros((NTOK, E), np.float16)
        va_used[: len(vu)] = (vt[vu].astype(np.float64) @ A + 0.5 * fb).astype(
            np.float16
        )
        pb_used = np.zeros((NTOK, E), np.float16)
        pb_used[: len(pu)] = (pt[pu].astype(np.float64) @ Bm + 0.5 * fb).astype(
            np.float16
        )
        in_maps.append(
            dict(
                common,
                va_used=va_used,
                pb_used=pb_used,
                v_idx=_wrap_idx(vinv),
                p_idx=_wrap_idx(pinv),
            )
        )
    return in_maps


def run(inputs, trace=False, tmpdir=None):
    if trace:
        _ensure_ntff_hook()
    in_maps = prep_in_maps(inputs)
    nc = _get_program()
    res = run_bass_kernel_spmd(
        nc,
        in_maps,
        core_ids=list(range(NCORES)),
        trace=trace,
        tmpdir=tmpdir,
    )
    out = np.concatenate(
        [res.results[k]["out"] for k in range(NCORES)], axis=0
    ).astype(np.float32)
    return out, res


def kernel(**inputs) -> np.ndarray:
    out, _ = run(inputs, trace=False)
    return out


# revision 8
# speedup vs baseline: 1.5932x; 1.4967x over previous
"""Code2Vec kernel for 8 Trainium2 NeuronCores.

Strategy (data-parallel over batch, fp16 data path):
  - Host folds the FC layer into the embedding tables:
      ctx @ fc_W.T = v1 @ (W1+W3).T + p @ W2.T
    so VA = value_table @ A + fc_b/2 and PB = path_table @ B + fc_b/2.
  - Host COMPACTS the tables per core: only the <=10240 rows a core's
    tokens reference are shipped (np.unique remap). The compacted row ids
    fit int16, which unlocks the ucode dma_gather instruction: one gpsimd
    instruction gathers 2560 rows (vs. 128 rows per indirect_dma_start),
    eliminating the ~1us/call SWDGE fixed cost that dominated the gather
    phase, and the 2.6MB footprint gives near-sequential HBM locality.
  - Everything flows in fp16 (tables, tanh, attention, tag matmul inputs,
    output store); PE matmuls run 4x faster than the fp32 2-pass path and
    the 41MB output write halves. PSUM accumulation stays fp32. Host
    upcasts the final output to fp32. Measured rel err ~6e-4 (gate 2e-2).
  - Attention pooling over R=20 is a PE matmul with 0/1 selection masks
    (compile-time pattern) as the stationary operand; an extra "es" column
    yields the softmax denominators in the same accumulation; 1/sum via
    DVE reciprocal so the scalar engine only ever needs the exp/tanh
    activation table set (no table reloads).
  - Tag classification: v^T (PE-transposed) @ tag_table^T (host-transposed)
    in fp16, softmax over 20000 via scalar-engine Exp (bias -10, fp16 out)
    with fused row-sum accumulation, DVE normalize, fp16 store.
"""

import numpy as np

import concourse.bass as bass
import concourse.bacc as bacc
import concourse.mybir as mybir
import concourse.tile as tile
from concourse.bass_utils import run_bass_kernel_spmd

NCORES = 8
B = 4096
R = 20
E = 128
TV = 20000
VV = 150000
PV = 200000

BL = B // NCORES         # 512 batch rows per core
NTOK = BL * R            # 10240 tokens per core
NTILE = NTOK // 128      # 80 token tiles
NB = BL // 128           # 4 b-tiles per core
TPB = R                  # 20 token tiles per b-tile
QTOK = NTOK // NB        # 2560 tokens per b-tile quarter
EC = 2048                # output columns per chunk
NEC = (TV + EC - 1) // EC  # 10 chunks (last = 1568)

F32 = mybir.dt.float32
F16 = mybir.dt.float16
I16 = mybir.dt.int16

IDXC = QTOK // 16        # 160 idx columns per gather call


def _body(nc, tc, aps):
    Alu = mybir.AluOpType
    Act = mybir.ActivationFunctionType
    va, pb, tagt, vidx, pidx, sel, attw, attb, ident, out = aps

    with (
        tc.tile_pool(name="const", bufs=1) as cpool,
    ):
        # ---- resident constants
        tag_sb = cpool.tile([128, TV], F16, tag="tag")
        nc.sync.dma_start(out=tag_sb[:], in_=tagt[:])
        sel_sb = cpool.tile([128, TPB * 128], F16, tag="sel")
        nc.sync.dma_start(out=sel_sb[:], in_=sel[:])
        attw_sb = cpool.tile([128, E], F16, tag="attw")
        nc.sync.dma_start(out=attw_sb[:], in_=attw[:])
        attb_sb = cpool.tile([128, 1], F32, tag="attb")
        nc.sync.dma_start(out=attb_sb[:], in_=attb[:])
        shift_sb = cpool.tile([128, 1], F32, tag="shift")
        nc.vector.memset(shift_sb[:], -10.0)
        ident_sb = cpool.tile([128, 128], F16, tag="ident")
        nc.sync.dma_start(out=ident_sb[:], in_=ident[:])
        vidx_sb = cpool.tile([128, NB * IDXC], I16, tag="vidx")
        nc.sync.dma_start(out=vidx_sb[:], in_=vidx[:])
        pidx_sb = cpool.tile([128, NB * IDXC], I16, tag="pidx")
        nc.sync.dma_start(out=pidx_sb[:], in_=pidx[:])

        vts = []

        # ---- stage A: gather + tanh + attention pool -> v^T per b-tile
        with (
            tc.tile_pool(name="gva", bufs=2) as gpool,
            tc.tile_pool(name="gpb", bufs=2) as hpool,
            tc.tile_pool(name="cbuf", bufs=2) as cbpool,
            tc.tile_pool(name="cwp", bufs=2) as cwpool,
            tc.tile_pool(name="small", bufs=2) as spool,
            tc.tile_pool(name="psv", bufs=2, space="PSUM") as psv,
            tc.tile_pool(name="psvt", bufs=2, space="PSUM") as psvt,
        ):
            for q in range(NB):
                g = gpool.tile([128, QTOK], F16, tag="g")
                h = hpool.tile([128, QTOK], F16, tag="h")
                # 4 SWDGE queues run their Q7 desc-gen ucode concurrently
                # (HW-probed 4x): pair each quarter's VA/PB on adjacent
                # queues so quarter q's gathers land in ceil((q+1)/2) rounds.
                nc.gpsimd.dma_gather(
                    g[:].rearrange("p (t e) -> p t e", e=128),
                    va,
                    vidx_sb[:, q * IDXC : (q + 1) * IDXC],
                    QTOK,
                    QTOK,
                    E,
                    single_packet=False,
                    queue_num=(2 * q) % 4,
                )
                nc.gpsimd.dma_gather(
                    h[:].rearrange("p (t e) -> p t e", e=128),
                    pb,
                    pidx_sb[:, q * IDXC : (q + 1) * IDXC],
                    QTOK,
                    QTOK,
                    E,
                    single_packet=False,
                    queue_num=(2 * q + 1) % 4,
                )
                c = cbpool.tile([128, QTOK], F16, tag="c")
                nc.vector.tensor_tensor(out=c[:], in0=g[:], in1=h[:], op=Alu.add)
                # c = tanh(VA_g + PB_g) in place
                nc.scalar.activation(out=c[:], in_=c[:], func=Act.Tanh)
                # scores_j = sum_e c[:, j, e] * att_w[e]
                scr = spool.tile([128, TPB], F32, tag="scr")
                ttrs = cbpool.tile([128, QTOK], F16, tag="ttrs")
                c3 = c[:].rearrange("p (g e) -> p g e", e=128)
                nc.vector.tensor_tensor(
                    out=ttrs[:].rearrange("p (g e) -> p g e", e=128),
                    in0=c3,
                    in1=attw_sb[:].rearrange("p e -> p () e").to_broadcast(
                        [128, TPB, 128]
                    ),
                    op=Alu.mult,
                )
                nc.vector.tensor_reduce(
                    out=scr[:],
                    in_=ttrs[:].rearrange("p (g e) -> p g e", e=128),
                    axis=mybir.AxisListType.X,
                    op=Alu.add,
                )
                es = spool.tile([128, TPB], F16, tag="es")
                nc.scalar.activation(
                    out=es[:], in_=scr[:], func=Act.Exp, bias=attb_sb[:, 0:1]
                )
                # cw[:, j, 0:128] = c * es ; cw[:, j, 128] = es
                cw = cwpool.tile([128, TPB * 129], F16, tag="cw")
                cw3 = cw[:].rearrange("p (g x) -> p g x", x=129)
                es3 = es[:].unsqueeze(2)
                nc.vector.tensor_tensor(
                    out=cw3[:, :, 0:128],
                    in0=c3,
                    in1=es3.to_broadcast([128, TPB, 128]),
                    op=Alu.mult,
                )
                nc.vector.tensor_copy(out=cw3[:, :, 128:129], in_=es3)

                pv = psv.tile([128, 129], F32, tag="pv")
                for jl in range(TPB):
                    nc.tensor.matmul(
                        out=pv[:],
                        lhsT=sel_sb[:, jl * 128 : (jl + 1) * 128],
                        rhs=cw3[:, jl, :],
                        start=(jl == 0),
                        stop=(jl == TPB - 1),
                    )
                rs = spool.tile([128, 1], F32, tag="rs")
                nc.vector.reciprocal(out=rs[:], in_=pv[:, 128:129])
                vsb = spool.tile([128, 128], F16, tag="vsb")
                nc.vector.tensor_scalar_mul(
                    out=vsb[:], in0=pv[:, 0:128], scalar1=rs[:]
                )
                pvt = psvt.tile([128, 128], F16, tag="pvt")
                nc.tensor.transpose(out=pvt[:], in_=vsb[:], identity=ident_sb[:])
                vt = cpool.tile([128, 128], F16, tag=f"vt{q}")
                nc.scalar.copy(out=vt[:], in_=pvt[:])
                vts.append(vt)

        # ---- stage C: logits = v^T.T @ tagT, softmax over 20000, fp16 store
        with (
            tc.tile_pool(name="ebuf", bufs=2) as epool,
            tc.tile_pool(name="stg", bufs=3) as stgpool,
            tc.tile_pool(name="sums", bufs=2) as supool,
            tc.tile_pool(name="psc", bufs=2, space="PSUM") as psc,
        ):
            for kb in range(NB):
                sums = supool.tile([128, NEC], F32, tag="sums")
                et = epool.tile([128, NEC * EC], F16, tag="et")
                for gi in range(NEC):
                    off = gi * EC
                    w = min(EC, TV - off)
                    pc = psc.tile([128, EC], F32, tag="pc")
                    for q in range(0, w, 512):
                        wq = min(512, w - q)
                        nc.tensor.matmul(
                            out=pc[:, q : q + wq],
                            lhsT=vts[kb][:],
                            rhs=tag_sb[:, off + q : off + q + wq],
                            start=True,
                            stop=True,
                        )
                    # fp16 exp with constant -10 logit shift (cancelled by the
                    # normalization): exp(l-10) <= e^7 fits fp16 range.
                    nc.scalar.activation(
                        out=et[:, off : off + w],
                        in_=pc[:, 0:w],
                        func=Act.Exp,
                        bias=shift_sb[:, 0:1],
                        accum_out=sums[:, gi : gi + 1],
                    )
                stot = supool.tile([128, 1], F32, tag="stot")
                nc.vector.reduce_sum(
                    out=stot[:], in_=sums[:], axis=mybir.AxisListType.X
                )
                rstot = supool.tile([128, 1], F32, tag="rstot")
                nc.vector.reciprocal(out=rstot[:], in_=stot[:])
                for gi in range(NEC):
                    off = gi * EC
                    w = min(EC, TV - off)
                    stg = stgpool.tile([128, EC], F16, tag="stg")
                    nc.vector.tensor_scalar_mul(
                        out=stg[:, 0:w], in0=et[:, off : off + w], scalar1=rstot[:]
                    )
                    nc.sync.dma_start(
                        out=out[kb * 128 : (kb + 1) * 128, off : off + w],
                        in_=stg[:, 0:w],
                    )


def _build_program():
    nc = bacc.Bacc(
        "TRN2",
        target_bir_lowering=False,
        debug=False,
        num_devices=NCORES,
        # SWDGE descriptor ring (SBUF carveout): dma_gather pushes one desc
        # pair per row; 64KB holds a 2560-row call's descriptors and is
        # probe-validated vs the 16KB default's overflow corruption.
        dynamic_dma_scratch_size=65536,
        num_swdge_queues=4,
    )
    aps = (
        nc.dram_tensor("va_used", [NTOK, E], F16, kind="ExternalInput").ap(),
        nc.dram_tensor("pb_used", [NTOK, E], F16, kind="ExternalInput").ap(),
        nc.dram_tensor("tag_t", [E, TV], F16, kind="ExternalInput").ap(),
        nc.dram_tensor("v_idx", [128, NB * IDXC], I16, kind="ExternalInput").ap(),
        nc.dram_tensor("p_idx", [128, NB * IDXC], I16, kind="ExternalInput").ap(),
        nc.dram_tensor("sel", [128, TPB * 128], F16, kind="ExternalInput").ap(),
        nc.dram_tensor("att_wb", [128, E], F16, kind="ExternalInput").ap(),
        nc.dram_tensor("att_bb", [128, 1], F32, kind="ExternalInput").ap(),
        nc.dram_tensor("ident", [128, 128], F16, kind="ExternalInput").ap(),
        nc.dram_tensor("out", [BL, TV], F16, kind="ExternalOutput").ap(),
    )
    with tile.TileContext(nc) as tc:
        _body(nc, tc, aps)
    nc.compile()
    return nc


_NC_CACHE = None


def _get_program():
    global _NC_CACHE
    if _NC_CACHE is None:
        _NC_CACHE = _build_program()
    return _NC_CACHE


def _install_neff_cache():
    """Cache compiled NEFFs by BIR hash — the stock bass_exec path recompiles
    (~6 min) on every fresh process even for an identical program."""
    import hashlib
    import os
    import shutil

    import concourse.bass2jax as b2j
    import concourse.bass_utils as bu

    if getattr(bu, "_c2v_neff_cache", False):
        return
    orig = bu.compile_bir_kernel

    def cached(bir_json, tmpdir, neff_name="file.neff"):
        h = hashlib.sha256(bir_json).hexdigest()[:24]
        cdir = os.path.expanduser("~/.c2v_neff_cache")
        os.makedirs(cdir, exist_ok=True)
        cpath = os.path.join(cdir, h + ".neff")
        dst = os.path.join(tmpdir, neff_name)
        if os.path.exists(cpath):
            shutil.copyfile(cpath, dst)
            return dst
        p = orig(bir_json, tmpdir, neff_name)
        try:
            shutil.copyfile(p, cpath)
        except OSError:
            pass
        return p

    bu.compile_bir_kernel = cached
    b2j.compile_bir_kernel = cached
    bu._c2v_neff_cache = True


_install_neff_cache()


def _ensure_ntff_hook():
    """The agent image's antenv lacks axon_hooks; recreate it via ctypes on
    the injected libaxon_pjrt.so so trace=True produces NTFF profiles."""
    import contextlib
    import ctypes
    import sys
    import types

    try:
        from antenv.axon_hooks import get_axon_ntff_profile_hook  # noqa: F401

        return
    except ImportError:
        pass

    so_path = "/opt/axon/libaxon_pjrt.so"
    lib = ctypes.CDLL(so_path)
    hook = None
    if hasattr(lib, "axon_start_nrt_profile"):
        lib.axon_start_nrt_profile.argtypes = [
            ctypes.POINTER(ctypes.c_int64),
            ctypes.c_size_t,
        ]
        lib.axon_start_nrt_profile.restype = ctypes.c_int64
        lib.axon_stop_nrt_profile.argtypes = [ctypes.c_char_p]
        lib.axon_stop_nrt_profile.restype = ctypes.c_int64

        @contextlib.contextmanager
        def _hook(output_dir, device_ids):
            import jax

            jax.devices()
            if device_ids:
                ids = (ctypes.c_int64 * len(device_ids))(*device_ids)
                rc = lib.axon_start_nrt_profile(ids, len(device_ids))
            else:
                rc = lib.axon_start_nrt_profile(None, 0)
            if rc != 0:
                raise RuntimeError(f"axon_start_nrt_profile rc={rc}")
            try:
                yield
            finally:
                n = lib.axon_stop_nrt_profile(str(output_dir).encode())
                print(f"ntff profile: {n} file(s) written to {output_dir}")

        hook = _hook

    mod = types.ModuleType("antenv.axon_hooks")
    mod._hook = hook
    mod.get_axon_ntff_profile_hook = lambda: mod._hook
    mod.set_axon_ntff_profile_hook = lambda h: setattr(mod, "_hook", h)
    sys.modules["antenv.axon_hooks"] = mod
    import antenv

    antenv.axon_hooks = mod


def _wrap_idx(idx):
    """[NTOK] int -> [128, NB*IDXC] int16 in dma_gather's wrapped layout:
    call q's token i lives at [i % 16, q*IDXC + i // 16], 16-row block
    replicated 8x down the partition dim."""
    blocks = []
    for q in range(NB):
        blk = idx[q * QTOK : (q + 1) * QTOK].reshape(IDXC, 16).T  # [16, IDXC]
        blocks.append(blk)
    one = np.concatenate(blocks, axis=1)  # [16, NB*IDXC]
    return np.ascontiguousarray(np.tile(one, (8, 1)).astype(np.int16))


def prep_in_maps(inputs):
    """Host-side input prep: fold FC into tables, compact tables to the rows
    each core actually uses (int16-indexable), transpose tag table, build
    selection masks; shard batch across cores."""
    v1i = np.asarray(inputs["value1_idx"]).astype(np.int64)
    pti = np.asarray(inputs["path_idx"]).astype(np.int64)
    vt = np.asarray(inputs["value_table"], dtype=np.float32)
    pt = np.asarray(inputs["path_table"], dtype=np.float32)
    tt = np.asarray(inputs["tag_table"], dtype=np.float32)
    fw = np.asarray(inputs["fc_W"], dtype=np.float64)
    fb = np.asarray(inputs["fc_b"], dtype=np.float64)
    aw = np.asarray(inputs["att_w"], dtype=np.float32)
    ab = np.float32(np.asarray(inputs["att_b"]))

    A = (fw[:, :E] + fw[:, 2 * E : 3 * E]).T  # [e_in, e_out]
    Bm = fw[:, E : 2 * E].T

    selm = np.zeros((128, TPB * 128), np.float16)
    p = np.arange(128)
    for j in range(TPB):
        bloc = (128 * j + p) // R
        selm[p, j * 128 + bloc] = 1.0

    common = dict(
        tag_t=np.ascontiguousarray(tt.T.astype(np.float16)),
        sel=selm,
        att_wb=np.ascontiguousarray(
            np.tile(aw[None, :].astype(np.float16), (128, 1))
        ),
        att_bb=np.full((128, 1), ab, np.float32),
        ident=np.eye(128, dtype=np.float16),
    )
    in_maps = []
    for k in range(NCORES):
        vtok = v1i[k * BL : (k + 1) * BL, :].reshape(-1)  # token t = b*R + r
        ptok = pti[k * BL : (k + 1) * BL, :].reshape(-1)
        vu, vinv = np.unique(vtok, return_inverse=True)
        pu, pinv = np.unique(ptok, return_inverse=True)
        va_used = np.zeros((NTOK, E), np.float16)
        va_used[: len(vu)] = (vt[vu].astype(np.float64) @ A + 0.5 * fb).astype(
            np.float16
        )
        pb_used = np.zeros((NTOK, E), np.float16)
        pb_used[: len(pu)] = (pt[pu].astype(np.float64) @ Bm + 0.5 * fb).astype(
            np.float16
        )
        in_maps.append(
            dict(
                common,
                va_used=va_used,
                pb_used=pb_used,
                v_idx=_wrap_idx(vinv),
                p_idx=_wrap_idx(pinv),
            )
        )
    return in_maps


def run(inputs, trace=False, tmpdir=None):
    if trace:
        _ensure_ntff_hook()
    in_maps = prep_in_maps(inputs)
    nc = _get_program()
    res = run_bass_kernel_spmd(
        nc,
        in_maps,
        core_ids=list(range(NCORES)),
        trace=trace,
        tmpdir=tmpdir,
    )
    out = np.concatenate(
        [res.results[k]["out"] for k in range(NCORES)], axis=0
    ).astype(np.float32)
    return out, res


def kernel(**inputs) -> np.ndarray:
    out, _ = run(inputs, trace=False)
    return out
